# revision 42
# baseline (speedup 1.0000x reference)
"""GQA attention kernel for Trainium2 (Bass/Tile), 8-core SPMD.

Problem: B=2, N=2048, DIM=1024, 16 query heads / 4 KV heads, head_dim=64, fp32.
Sharding: core c = (batch b=c//4, kv-group g=c%4). Each core computes its
group's 4 query heads + 1 shared KV head over the full sequence, and a partial
output projection (its 256 rows of Wo). Host sums the 4 group partials per
batch and adds the bias.

Data path (all bf16 on PE, fp32 in PSUM):
  - Host pre-transposes x -> xT [1024, 2048] bf16 per batch, slices weights.
  - Projections: Q^T [256(2 head pairs), N], K^T [64, N] (duplicated onto
    both partition halves for head-pair score matmuls), V natural [N, 64]
    stored as vones [128, 16, 65] with an all-ones column for row-sums.
  - Scores S^T [128 keys, q] = K^T-tile (stationary) x Q^T (moving); exp on
    ACT into P [keys, q] bf16 (no max subtraction; |s| small by construction).
  - PV: P-tile [keys, 128 q] stationary x vones [keys, 65] moving ->
    psum [128 q, 65] accumulated over 16 key tiles; col 64 = sum of exp.
  - Normalize with per-partition scalar multiply (DVE), pack A [tok, 256],
    PE-transpose to A^T, out-proj out^T[1024, N] = Wo-tile x A^T.

The q dimension is processed in two mega-chunks of 1024 so exp units are
[128,1024] (2 psum banks, double buffered = 4) + 2 PV banks + 2 outproj/
transpose banks = 8 banks total.
"""

import sys

if "/opt/trn_rl_repo" not in sys.path:
    sys.path.insert(0, "/opt/trn_rl_repo")

from contextlib import ExitStack

import numpy as np

import concourse.bass as bass
import concourse.mybir as mybir
import concourse.tile as tile
from concourse import bacc, bass_utils
from concourse.bass import ds, ts
from concourse.masks import make_identity

F32 = mybir.dt.float32
BF16 = mybir.dt.bfloat16
EXPF = mybir.ActivationFunctionType.Exp

DIM = 1024
D = 64  # head dim
SCALE = D ** -0.5


def build_nc(NSEQ=2048):
    KB = NSEQ // 128          # key tiles
    NC_CH = NSEQ // 512       # 512-token chunks
    MCS = [(0, NSEQ // 2), (NSEQ // 2, NSEQ // 2)]  # (q0, qw) mega-chunks

    nc = bacc.Bacc("TRN2", target_bir_lowering=False, debug=False)
    xT = nc.dram_tensor("xT", [DIM, NSEQ], BF16, kind="ExternalInput").ap()
    wq = nc.dram_tensor("wq", [DIM, 256], BF16, kind="ExternalInput").ap()
    wk = nc.dram_tensor("wk", [DIM, D], BF16, kind="ExternalInput").ap()
    wv = nc.dram_tensor("wv", [DIM, D], BF16, kind="ExternalInput").ap()
    wo = nc.dram_tensor("wo", [256, DIM], BF16, kind="ExternalInput").ap()
    out = nc.dram_tensor("out", [DIM, NSEQ], BF16, kind="ExternalOutput").ap()
    # pair1 contribution to mc1's out-proj; summed into out on the host
    out1 = nc.dram_tensor("out1", [DIM, NSEQ // 2], BF16,
                          kind="ExternalOutput").ap()

    with tile.TileContext(nc) as tc, ExitStack() as ctx:
        sb = ctx.enter_context(tc.tile_pool(name="sb", bufs=1))
        dyn = ctx.enter_context(tc.tile_pool(name="dyn", bufs=1))
        ps = ctx.enter_context(tc.tile_pool(name="ps", bufs=1, space="PSUM"))

        # ---- persistent SBUF ----
        wq_sb = sb.tile([128, 8, 256], BF16)
        wk_sb = sb.tile([128, 8, D], BF16)
        wv_sb = sb.tile([128, 8, D], BF16)
        wo_sb = sb.tile([128, 2, DIM], BF16)
        ident = sb.tile([128, 128], BF16)
        wrow = sb.tile([128, 128], BF16)
        xt_sb = sb.tile([128, 8, NSEQ], BF16)
        QT = sb.tile([128, 2, NSEQ], BF16)
        KT = sb.tile([128, NSEQ], BF16)
        vones = sb.tile([128, KB, D + 1], BF16)
        AT = sb.tile([128, 2, NSEQ], BF16)
        warm = sb.tile([128, 1], F32)
        wone = sb.tile([128, 1], BF16)

        # ---- weight / x DMAs (emission order = DMA device order) ----
        # first kb of scores needs x cols 0:1024 + wq + wk as early as
        # possible; stream x in 256-col pieces so Q-proj starts early
        xr = xT.rearrange("(t p) m -> p t m", p=128)
        nc.sync.dma_start(out=wq_sb, in_=wq.rearrange("(t p) m -> p t m", p=128))
        nc.sync.dma_start(out=xt_sb[:, :, 0:256], in_=xr[:, :, 0:256])
        nc.sync.dma_start(out=xt_sb[:, :, 256:512], in_=xr[:, :, 256:512])
        nc.sync.dma_start(out=wk_sb, in_=wk.rearrange("(t p) m -> p t m", p=128))
        for c in range(2, 8):
            nc.sync.dma_start(out=xt_sb[:, :, ts(c, 256)], in_=xr[:, :, ts(c, 256)])
        nc.sync.dma_start(out=wv_sb, in_=wv.rearrange("(t p) m -> p t m", p=128))
        nc.sync.dma_start(out=wo_sb, in_=wo.rearrange("(t p) m -> p t m", p=128))
        make_identity(nc, ident)
        nc.vector.memset(wone, 1.0)
        nc.vector.memset(wrow, 1.0)
        nc.vector.memset(vones, 1.0)
        nc.scalar.activation(out=warm, in_=wone, func=EXPF, scale=1.0)

        # keep PE continuously busy until the first x/wq DMAs land so real
        # matmuls start at the ramped clock (p-state) instead of mid speed
        warm_n = [0]

        def emit_warm(n):
            for _ in range(n):
                w = warm_n[0]
                warm_n[0] += 1
                pw = ps.tile([128, 512], F32, tag="op", bufs=2,
                             name=f"pwarm{w}")
                nc.tensor.matmul(pw[0:1, 0:128], wone[0:1, 0:1], wrow[0:1, :],
                                 start=True, stop=True)

        emit_warm(40)

        # ---- prologue emitters (128-col pieces so PE blocks stay ~430ns
        # and never starve the ACT exp stream) ----
        def emit_qproj(j, p):
            """Q^T head pair p for 128-token piece j -> QT[:, p, j*128:+128]."""
            pq = ps.tile([128, 1024], F32, tag="sp", bufs=2, name=f"pq{j}_{p}")
            for t in range(8):
                nc.tensor.matmul(pq[:, 0:128], wq_sb[:, t, ts(p, 128)],
                                 xt_sb[:, t, ts(j, 128)],
                                 start=(t == 0), stop=(t == 7))
            nc.vector.tensor_copy(QT[:, p, ts(j, 128)], pq[:, 0:128])

        def emit_kproj(j):
            """K^T for 128-token piece j -> KT[0:64, j*128:+128]."""
            pk = ps.tile([128, 512], F32, tag="pv", bufs=2, name=f"pk{j}")
            for t in range(8):
                nc.tensor.matmul(pk[0:64, 0:128], wk_sb[:, t, :],
                                 xt_sb[:, t, ts(j, 128)],
                                 start=(t == 0), stop=(t == 7))
            nc.vector.tensor_copy(KT[0:64, ts(j, 128)], pk[0:64, 0:128])

        def emit_vproj(tb):
            """V natural for token tile tb -> vones[:, tb, 0:64]."""
            pv = ps.tile([128, 512], F32, tag="op", bufs=2, name=f"pvv{tb}")
            for t in range(8):
                nc.tensor.matmul(pv[:, 0:D],
                                 xt_sb[:, t, ts(tb, 128)], wv_sb[:, t, :],
                                 start=(t == 0), stop=(t == 7))
            nc.vector.tensor_copy(vones[:, tb, 0:D], pv[:, 0:D])

        def emit_ktdup():
            # duplicate K^T onto partitions 64:128 (SBUF->SBUF DMA crosses
            # partitions; engines cannot)
            nc.sync.dma_start(out=KT[64:128, :], in_=KT[0:64, :])

        # ---- attention state ----
        P_of = {}      # window -> P tile
        A_of = {}      # mci -> A tile

        def emit_scores_exp(mci, h, kb):
            q0, qw = MCS[mci]
            p, i = h // 2, h % 2
            sp = ps.tile([128, 1024], F32, tag="sp", bufs=2,
                         name=f"sp{mci}_{h}_{kb}")
            for j in range(qw // 512):
                nc.tensor.matmul(sp[:, ts(j, 512)],
                                 KT[ds(i * D, D), ts(kb, 128)],
                                 QT[ds(i * D, D), p, ds(q0 + j * 512, 512)],
                                 start=True, stop=True)
            nc.scalar.activation(out=P_of[(mci, h)][:, kb, 0:qw],
                                 in_=sp[:, 0:qw], func=EXPF, scale=SCALE)

        def emit_scores_exp_half(mci, h, kb, j):
            """512-wide score+exp unit; used in the first window so ACT can
            start on Q^T cols 0:512 before the rest of x has landed."""
            q0, qw = MCS[mci]
            p, i = h // 2, h % 2
            sp = ps.tile([128, 1024], F32, tag="sp", bufs=2,
                         name=f"sph{mci}_{h}_{kb}_{j}")
            nc.tensor.matmul(sp[:, 0:512],
                             KT[ds(i * D, D), ts(kb, 128)],
                             QT[ds(i * D, D), p, ds(q0 + j * 512, 512)],
                             start=True, stop=True)
            nc.scalar.activation(out=P_of[(mci, h)][:, kb, ts(j, 512)],
                                 in_=sp[:, 0:512], func=EXPF, scale=SCALE)

        def emit_pv(mci, h, qb, tag="pv", chain="dve"):
            """PV for query block qb (128 tokens) of head h, mega-chunk mci;
            includes normalize into A and (for odd heads) the A^T transpose.
            chain="act" moves the normalize-mul and A^T copy onto the ACT
            engine (used in the tail where ACT is otherwise idle)."""
            q0, qw = MCS[mci]
            P = P_of[(mci, h)]
            acc = ps.tile([128, 512] if tag != "sp" else [128, 1024], F32,
                          tag=tag, bufs=2, name=f"acc{mci}_{h}_{qb}")
            for kb in range(KB):
                nc.tensor.matmul(acc[:, 0:D + 1], P[:, kb, ds(qb * 128, 128)],
                                 vones[:, kb, :],
                                 start=(kb == 0), stop=(kb == KB - 1))
            rc = dyn.tile([128, 1], F32, tag="rc", bufs=6,
                          name=f"rc{mci}_{h}_{qb}")
            nc.vector.reciprocal(out=rc, in_=acc[:, D:D + 1])
            adst = A_of[mci][:, qb, ds(h * D, D)]
            if chain == "act":
                nc.scalar.activation(out=adst, in_=acc[:, 0:D],
                                     func=mybir.ActivationFunctionType.Copy,
                                     scale=rc)
            else:
                nc.vector.tensor_scalar_mul(adst, acc[:, 0:D], rc)
            if h % 2 == 1:
                pr = h // 2
                tp = ps.tile([128, 512], BF16, tag="op", bufs=2,
                             name=f"tp{mci}_{h}_{qb}")
                nc.tensor.transpose(tp[:, 0:128], A_of[mci][:, qb, ts(pr, 128)],
                                    ident)
                atdst = AT[:, pr, ds(q0 + qb * 128, 128)]
                if chain == "act":
                    nc.scalar.copy(atdst, tp[:, 0:128])
                else:
                    nc.vector.tensor_copy(atdst, tp[:, 0:128])

        def emit_outproj(mci, ct, tch, act_copy=False, tag="op"):
            """out^T[ct*128:+128, q0+tch*512:+512]."""
            q0, qw = MCS[mci]
            po = ps.tile([128, 512] if tag != "sp" else [128, 1024], F32,
                         tag=tag, bufs=2, name=f"po{mci}_{ct}_{tch}")
            pv512 = po[:, 0:512]
            for ft in range(2):
                nc.tensor.matmul(pv512, wo_sb[:, ft, ts(ct, 128)],
                                 AT[:, ft, ds(q0 + tch * 512, 512)],
                                 start=(ft == 0), stop=(ft == 1))
            ot = dyn.tile([128, 512], BF16, tag="os", bufs=6,
                          name=f"ot{mci}_{ct}_{tch}")
            if act_copy:
                nc.scalar.copy(ot, pv512)
            else:
                nc.vector.tensor_copy(ot, pv512)
            nc.sync.dma_start(out=out[ts(ct, 128), ds(q0 + tch * 512, 512)],
                              in_=ot)

        # mc1 out-proj is split by head pair: ft0 groups run inside windows
        # 6-7 (pair0 of mc1 is normalized by then) and DMA with bypass; ft1
        # groups run in the tail and accumulate into DRAM (AluOpType.add)
        ocp = out.rearrange("(cp c p) m -> cp p c m", c=2, p=128)
        ocp1 = out1.rearrange("(cp c p) m -> cp p c m", c=2, p=128)
        og_i = [0]

        def emit_og(ft, tch, cp):
            ot2 = dyn.tile([128, 2, 512], BF16, tag="os2", bufs=8,
                           name=f"ot2_{ft}_{tch}_{cp}")
            for cti in range(2):
                i = og_i[0]
                og_i[0] += 1
                tg = "op" if i % 2 == 0 else "sp"
                po = ps.tile([128, 512] if tg == "op" else [128, 1024], F32,
                             tag=tg, bufs=2, name=f"tpo{ft}_{tch}_{cp}_{cti}")
                pv512 = po[:, 0:512]
                nc.tensor.matmul(pv512, wo_sb[:, ft, ts(2 * cp + cti, 128)],
                                 AT[:, ft, ds(1024 + tch * 512, 512)],
                                 start=True, stop=True)
                dst = ot2[:, cti, :]
                if i % 2 == 0:
                    nc.scalar.copy(dst, pv512)
                else:
                    nc.vector.tensor_copy(dst, pv512)
            dv = ocp if ft == 0 else ocp1
            off = 1024 if ft == 0 else 0
            nc.sync.dma_start(out=dv[cp][:, :, ds(off + tch * 512, 512)],
                              in_=ot2)

        # ---- window schedule with sprinkled side-work ----
        windows = [(mci, h) for mci in range(2) for h in range(4)]

        # pre-window prologue: Q pair0 pieces 0..3 (cols 0:512, the first
        # half-units' moving data) and K piece 0 (keys 0:128); warm matmuls
        # bridge the x-piece DMA arrivals so the PE clock stays ramped
        emit_qproj(0, 0)
        emit_qproj(1, 0)
        emit_warm(18)
        emit_qproj(2, 0)
        emit_qproj(3, 0)
        emit_warm(6)
        emit_kproj(0)
        emit_warm(4)

        # per-window sprinkle thunks: (cost_ns, fn); kept <=~450ns each so
        # they slot into the PE slack between scores units
        QP = 430
        PV = 450
        OP = 450
        VP = 220

        def sprinkles(wi):
            mci, h = windows[wi]
            th = []
            if wi == 0:
                # kproj/ktdup are emitted inline with the units (ordering
                # constraint: PE is in-order, a score matmul emitted before
                # its K piece would stall the whole PE stream)
                th += [(VP, lambda tb=tb: emit_vproj(tb)) for tb in range(12)]
            else:
                pmci, ph = windows[wi - 1]
                nqb = MCS[pmci][1] // 128
                pv_th = [(PV, lambda qb=qb, m=pmci, hh=ph: emit_pv(m, hh, qb))
                         for qb in range(nqb)]
                if wi == 1:
                    # V tiles 12..15 must land before the first PV thunk
                    th += [(VP, lambda tb=tb: emit_vproj(tb))
                           for tb in range(12, 16)]
                    th += pv_th
                    th += [(QP, lambda j=j: emit_qproj(j, 1))
                           for j in range(8)]
                else:
                    th += pv_th
                if wi == 2:
                    th += [(QP, lambda j=j: emit_qproj(j, 0))
                           for j in range(8, 12)]
                    th += [(QP, lambda j=j: emit_qproj(j, 1))
                           for j in range(8, 12)]
                elif wi == 3:
                    th += [(QP, lambda j=j: emit_qproj(j, 0))
                           for j in range(12, 16)]
                    th += [(QP, lambda j=j: emit_qproj(j, 1))
                           for j in range(12, 16)]
                if wi in (5, 6):  # out-proj of mc0 during (1,h1)/(1,h2)
                    units = [(ct, tch) for tch in range(2) for ct in range(8)]
                    part = units[(wi - 5) * 8:(wi - 4) * 8]
                    th += [(OP, lambda u=u: emit_outproj(0, u[0], u[1]))
                           for u in part]
                if wi == 6:  # pair0 of mc1 out-proj, tch0 (query blocks 0-3)
                    th += [(OP, lambda cp=cp: emit_og(0, 0, cp))
                           for cp in range(4)]
                elif wi == 7:  # pair0 of mc1 out-proj, tch1
                    th += [(OP, lambda cp=cp: emit_og(0, 1, cp))
                           for cp in range(4)]
            return th

        # first-window unit order: the first 4 key tiles run as 512-wide
        # halves (the a-halves only need Q^T cols 0:512, i.e. the first two
        # x DMA pieces) so ACT starts early; the rest are full units, paced
        # well behind the x DMA stream
        W0_UNITS = ([(kb, 0) for kb in range(4)] + [(kb, 1) for kb in range(4)]
                    + [(kb, None) for kb in range(4, 16)])

        kp_done = set([0])
        qp1_done = [False]

        for wi, (mci, h) in enumerate(windows):
            P_of[(mci, h)] = dyn.tile([128, KB, 1024], BF16, tag="P", bufs=2,
                                      name=f"P{mci}_{h}")
            if h == 1:
                A_of[mci] = dyn.tile([128, MCS[mci][1] // 128, 256], BF16,
                                     tag="A", bufs=2, name=f"A{mci}")
            th = sprinkles(wi)
            total = sum(c for c, _ in th)
            spent = 0
            units = W0_UNITS if wi == 0 else [(kb, None) for kb in range(KB)]
            NU = len(units)
            for ui, (kb, half) in enumerate(units):
                if wi == 0:
                    if kb not in kp_done:
                        emit_kproj(kb)
                        kp_done.add(kb)
                        if len(kp_done) == KB:
                            emit_ktdup()
                    if (half == 1 or half is None) and not qp1_done[0]:
                        for j in range(4, 8):
                            emit_qproj(j, 0)
                        qp1_done[0] = True
                if half is None:
                    emit_scores_exp(mci, h, kb)
                else:
                    emit_scores_exp_half(mci, h, kb, half)
                # spread side work across the window, ~even by cost; finish
                # well before the window ends: the next window's first exp
                # WAR-waits on the previous PV reads, so late thunks stall ACT
                goal = min(total, total * (ui + 1) // min(NU - 2, 11))
                while th and spent < goal:
                    c, f = th.pop(0)
                    f()
                    spent += c
            while th:
                th.pop(0)[1]()

        # ---- tail: PV of last head interleaved with the pair1 half of
        # mc1's out-proj, which accumulates into DRAM on top of the pair0
        # half that was DMA'd out during windows 6-7.
        def tail_pv(qb, tag):
            emit_pv(1, 3, qb, tag=tag, chain="act" if qb % 2 == 0 else "dve")

        tail_pv(0, "pv")
        tail_pv(1, "pv")
        tail_pv(2, "sp")
        tail_pv(3, "sp")
        emit_og(1, 0, 0)
        emit_og(1, 0, 1)
        tail_pv(4, "pv")
        tail_pv(5, "pv")
        emit_og(1, 0, 2)
        emit_og(1, 0, 3)
        tail_pv(6, "sp")
        tail_pv(7, "sp")
        for cp in range(4):
            emit_og(1, 1, cp)

    nc.compile()
    return nc


_CACHE = {}


def _get_nc(NSEQ):
    if NSEQ not in _CACHE:
        _CACHE[NSEQ] = build_nc(NSEQ)
    return _CACHE[NSEQ]


def kernel(x, Wq, Wk, Wv, Wo, bo):
    """Full-input entry point: shard over 8 cores, run, gather."""
    import ml_dtypes
    bf16 = ml_dtypes.bfloat16
    x = np.asarray(x, np.float32)
    Wq = np.asarray(Wq, np.float32)
    Wk = np.asarray(Wk, np.float32)
    Wv = np.asarray(Wv, np.float32)
    Wo = np.asarray(Wo, np.float32)
    bo = np.asarray(bo, np.float32)
    B, N, C = x.shape
    nc = _get_nc(N)
    in_maps = []
    for c in range(8):
        b, g = c // 4, c % 4
        in_maps.append({
            "xT": np.ascontiguousarray(x[b].T).astype(bf16),
            "wq": np.ascontiguousarray(Wq[:, g * 256:(g + 1) * 256]).astype(bf16),
            "wk": np.ascontiguousarray(Wk[:, g * D:(g + 1) * D]).astype(bf16),
            "wv": np.ascontiguousarray(Wv[:, g * D:(g + 1) * D]).astype(bf16),
            "wo": np.ascontiguousarray(Wo[g * 256:(g + 1) * 256, :]).astype(bf16),
        })
    res = bass_utils.run_bass_kernel_spmd(nc, in_maps, core_ids=list(range(8)))
    outs = []
    for c in range(8):
        o = np.asarray(res.results[c]["out"]).astype(np.float32)
        o[:, N // 2:] += np.asarray(res.results[c]["out1"]).astype(np.float32)
        outs.append(o)
    full = np.empty((B, N, C), np.float32)
    for b in range(B):
        acc = outs[4 * b]
        for g in range(1, 4):
            acc = acc + outs[4 * b + g]
        full[b] = acc.T + bo[None, :]
    return full


# revision 43
# speedup vs baseline: 1.0179x; 1.0179x over previous
"""GQA attention kernel for Trainium2 (Bass/Tile), 8-core SPMD.

Problem: B=2, N=2048, DIM=1024, 16 query heads / 4 KV heads, head_dim=64, fp32.
Sharding: core c = (batch b=c//4, kv-group g=c%4). Each core computes its
group's 4 query heads + 1 shared KV head over the full sequence, and a partial
output projection (its 256 rows of Wo). Host sums the 4 group partials per
batch and adds the bias.

Data path (all bf16 on PE, fp32 in PSUM):
  - Host pre-transposes x -> xT [1024, 2048] bf16 per batch, slices weights.
  - Projections: Q^T [256(2 head pairs), N], K^T [64, N] (duplicated onto
    both partition halves for head-pair score matmuls), V natural [N, 64]
    stored as vones [128, 16, 65] with an all-ones column for row-sums.
  - Scores S^T [128 keys, q] = K^T-tile (stationary) x Q^T (moving); exp on
    ACT into P [keys, q] bf16 (no max subtraction; |s| small by construction).
  - PV: P-tile [keys, 128 q] stationary x vones [keys, 65] moving ->
    psum [128 q, 65] accumulated over 16 key tiles; col 64 = sum of exp.
  - Normalize with per-partition scalar multiply (DVE), pack A [tok, 256],
    PE-transpose to A^T, out-proj out^T[1024, N] = Wo-tile x A^T.

The q dimension is processed in two mega-chunks of 1024 so exp units are
[128,1024] (2 psum banks, double buffered = 4) + 2 PV banks + 2 outproj/
transpose banks = 8 banks total.
"""

import sys

if "/opt/trn_rl_repo" not in sys.path:
    sys.path.insert(0, "/opt/trn_rl_repo")

from contextlib import ExitStack

import numpy as np

import concourse.bass as bass
import concourse.mybir as mybir
import concourse.tile as tile
from concourse import bacc, bass_utils
from concourse.bass import ds, ts
from concourse.masks import make_identity

F32 = mybir.dt.float32
BF16 = mybir.dt.bfloat16
EXPF = mybir.ActivationFunctionType.Exp

DIM = 1024
D = 64  # head dim
SCALE = D ** -0.5


def build_nc(NSEQ=2048):
    KB = NSEQ // 128          # key tiles
    NC_CH = NSEQ // 512       # 512-token chunks
    MCS = [(0, NSEQ // 2), (NSEQ // 2, NSEQ // 2)]  # (q0, qw) mega-chunks

    nc = bacc.Bacc("TRN2", target_bir_lowering=False, debug=False)
    xT = nc.dram_tensor("xT", [DIM, NSEQ], BF16, kind="ExternalInput").ap()
    wq = nc.dram_tensor("wq", [DIM, 256], BF16, kind="ExternalInput").ap()
    wk = nc.dram_tensor("wk", [DIM, D], BF16, kind="ExternalInput").ap()
    wv = nc.dram_tensor("wv", [DIM, D], BF16, kind="ExternalInput").ap()
    wo = nc.dram_tensor("wo", [256, DIM], BF16, kind="ExternalInput").ap()
    out = nc.dram_tensor("out", [DIM, NSEQ], BF16, kind="ExternalOutput").ap()
    # pair1 contribution to mc1's out-proj; summed into out on the host
    out1 = nc.dram_tensor("out1", [DIM, NSEQ // 2], BF16,
                          kind="ExternalOutput").ap()

    with tile.TileContext(nc) as tc, ExitStack() as ctx:
        sb = ctx.enter_context(tc.tile_pool(name="sb", bufs=1))
        dyn = ctx.enter_context(tc.tile_pool(name="dyn", bufs=1))
        ps = ctx.enter_context(tc.tile_pool(name="ps", bufs=1, space="PSUM"))

        # ---- persistent SBUF ----
        wq_sb = sb.tile([128, 8, 256], BF16)
        wk_sb = sb.tile([128, 8, D], BF16)
        wv_sb = sb.tile([128, 8, D], BF16)
        wo_sb = sb.tile([128, 2, DIM], BF16)
        ident = sb.tile([128, 128], BF16)
        wrow = sb.tile([128, 128], BF16)
        xt_sb = sb.tile([128, 8, NSEQ], BF16)
        QT = sb.tile([128, 2, NSEQ], BF16)
        KT = sb.tile([128, NSEQ], BF16)
        vones = sb.tile([128, KB, D + 1], BF16)
        AT = sb.tile([128, 2, NSEQ], BF16)
        warm = sb.tile([128, 1], F32)
        wone = sb.tile([128, 1], BF16)

        # ---- weight / x DMAs (emission order = DMA device order) ----
        # first kb of scores needs x cols 0:1024 + wq + wk as early as
        # possible; stream x in 256-col pieces so Q-proj starts early
        xr = xT.rearrange("(t p) m -> p t m", p=128)
        nc.sync.dma_start(out=wq_sb, in_=wq.rearrange("(t p) m -> p t m", p=128))
        nc.sync.dma_start(out=xt_sb[:, :, 0:256], in_=xr[:, :, 0:256])
        nc.sync.dma_start(out=xt_sb[:, :, 256:512], in_=xr[:, :, 256:512])
        nc.sync.dma_start(out=wk_sb, in_=wk.rearrange("(t p) m -> p t m", p=128))
        for c in range(2, 8):
            nc.sync.dma_start(out=xt_sb[:, :, ts(c, 256)], in_=xr[:, :, ts(c, 256)])
        nc.sync.dma_start(out=wv_sb, in_=wv.rearrange("(t p) m -> p t m", p=128))
        nc.sync.dma_start(out=wo_sb, in_=wo.rearrange("(t p) m -> p t m", p=128))
        make_identity(nc, ident)
        nc.vector.memset(wone, 1.0)
        nc.vector.memset(wrow, 1.0)
        nc.vector.memset(vones, 1.0)
        nc.scalar.activation(out=warm, in_=wone, func=EXPF, scale=1.0)

        # keep PE continuously busy until the first x/wq DMAs land so real
        # matmuls start at the ramped clock (p-state) instead of mid speed
        warm_n = [0]

        def emit_warm(n):
            for _ in range(n):
                w = warm_n[0]
                warm_n[0] += 1
                pw = ps.tile([128, 512], F32, tag="op", bufs=2,
                             name=f"pwarm{w}")
                nc.tensor.matmul(pw[0:1, 0:128], wone[0:1, 0:1], wrow[0:1, :],
                                 start=True, stop=True)

        emit_warm(40)

        # ---- prologue emitters (128-col pieces so PE blocks stay ~430ns
        # and never starve the ACT exp stream) ----
        def emit_qproj(j, p):
            """Q^T head pair p for 128-token piece j -> QT[:, p, j*128:+128]."""
            pq = ps.tile([128, 1024], F32, tag="sp", bufs=2, name=f"pq{j}_{p}")
            for t in range(8):
                nc.tensor.matmul(pq[:, 0:128], wq_sb[:, t, ts(p, 128)],
                                 xt_sb[:, t, ts(j, 128)],
                                 start=(t == 0), stop=(t == 7))
            nc.vector.tensor_copy(QT[:, p, ts(j, 128)], pq[:, 0:128])

        def emit_kproj(j):
            """K^T for 128-token piece j -> KT[0:64, j*128:+128]."""
            pk = ps.tile([128, 512], F32, tag="pv", bufs=2, name=f"pk{j}")
            for t in range(8):
                nc.tensor.matmul(pk[0:64, 0:128], wk_sb[:, t, :],
                                 xt_sb[:, t, ts(j, 128)],
                                 start=(t == 0), stop=(t == 7))
            nc.vector.tensor_copy(KT[0:64, ts(j, 128)], pk[0:64, 0:128])

        def emit_vproj(tb):
            """V natural for token tile tb -> vones[:, tb, 0:64]."""
            pv = ps.tile([128, 512], F32, tag="op", bufs=2, name=f"pvv{tb}")
            for t in range(8):
                nc.tensor.matmul(pv[:, 0:D],
                                 xt_sb[:, t, ts(tb, 128)], wv_sb[:, t, :],
                                 start=(t == 0), stop=(t == 7))
            nc.vector.tensor_copy(vones[:, tb, 0:D], pv[:, 0:D])

        def emit_ktdup():
            # duplicate K^T onto partitions 64:128 (SBUF->SBUF DMA crosses
            # partitions; engines cannot)
            nc.sync.dma_start(out=KT[64:128, :], in_=KT[0:64, :])

        # ---- attention state ----
        P_of = {}      # window -> P tile
        A_of = {}      # mci -> A tile

        def emit_scores_exp(mci, h, kb):
            q0, qw = MCS[mci]
            p, i = h // 2, h % 2
            sp = ps.tile([128, 1024], F32, tag="sp", bufs=2,
                         name=f"sp{mci}_{h}_{kb}")
            for j in range(qw // 512):
                nc.tensor.matmul(sp[:, ts(j, 512)],
                                 KT[ds(i * D, D), ts(kb, 128)],
                                 QT[ds(i * D, D), p, ds(q0 + j * 512, 512)],
                                 start=True, stop=True)
            nc.scalar.activation(out=P_of[(mci, h)][:, kb, 0:qw],
                                 in_=sp[:, 0:qw], func=EXPF, scale=SCALE)

        def emit_scores_exp_half(mci, h, kb, j):
            """512-wide score+exp unit; used in the first window so ACT can
            start on Q^T cols 0:512 before the rest of x has landed."""
            q0, qw = MCS[mci]
            p, i = h // 2, h % 2
            sp = ps.tile([128, 1024], F32, tag="sp", bufs=2,
                         name=f"sph{mci}_{h}_{kb}_{j}")
            nc.tensor.matmul(sp[:, 0:512],
                             KT[ds(i * D, D), ts(kb, 128)],
                             QT[ds(i * D, D), p, ds(q0 + j * 512, 512)],
                             start=True, stop=True)
            nc.scalar.activation(out=P_of[(mci, h)][:, kb, ts(j, 512)],
                                 in_=sp[:, 0:512], func=EXPF, scale=SCALE)

        def emit_pv(mci, h, qb, tag="pv", chain="dve"):
            """PV for query block qb (128 tokens) of head h, mega-chunk mci;
            includes normalize into A and (for odd heads) the A^T transpose.
            chain="act" moves the normalize-mul and A^T copy onto the ACT
            engine (used in the tail where ACT is otherwise idle)."""
            q0, qw = MCS[mci]
            P = P_of[(mci, h)]
            acc = ps.tile([128, 512] if tag != "sp" else [128, 1024], F32,
                          tag=tag, bufs=2, name=f"acc{mci}_{h}_{qb}")
            for kb in range(KB):
                nc.tensor.matmul(acc[:, 0:D + 1], P[:, kb, ds(qb * 128, 128)],
                                 vones[:, kb, :],
                                 start=(kb == 0), stop=(kb == KB - 1))
            rc = dyn.tile([128, 1], F32, tag="rc", bufs=6,
                          name=f"rc{mci}_{h}_{qb}")
            nc.vector.reciprocal(out=rc, in_=acc[:, D:D + 1])
            adst = A_of[mci][:, qb, ds(h * D, D)]
            if chain == "act":
                nc.scalar.activation(out=adst, in_=acc[:, 0:D],
                                     func=mybir.ActivationFunctionType.Copy,
                                     scale=rc)
            else:
                nc.vector.tensor_scalar_mul(adst, acc[:, 0:D], rc)
            if h % 2 == 1:
                pr = h // 2
                tp = ps.tile([128, 512], BF16, tag="op", bufs=2,
                             name=f"tp{mci}_{h}_{qb}")
                nc.tensor.transpose(tp[:, 0:128], A_of[mci][:, qb, ts(pr, 128)],
                                    ident)
                atdst = AT[:, pr, ds(q0 + qb * 128, 128)]
                if chain == "act":
                    nc.scalar.copy(atdst, tp[:, 0:128])
                else:
                    nc.vector.tensor_copy(atdst, tp[:, 0:128])

        def emit_outproj(mci, ct, tch, act_copy=False, tag="op"):
            """out^T[ct*128:+128, q0+tch*512:+512]."""
            q0, qw = MCS[mci]
            po = ps.tile([128, 512] if tag != "sp" else [128, 1024], F32,
                         tag=tag, bufs=2, name=f"po{mci}_{ct}_{tch}")
            pv512 = po[:, 0:512]
            for ft in range(2):
                nc.tensor.matmul(pv512, wo_sb[:, ft, ts(ct, 128)],
                                 AT[:, ft, ds(q0 + tch * 512, 512)],
                                 start=(ft == 0), stop=(ft == 1))
            ot = dyn.tile([128, 512], BF16, tag="os", bufs=6,
                          name=f"ot{mci}_{ct}_{tch}")
            if act_copy:
                nc.scalar.copy(ot, pv512)
            else:
                nc.vector.tensor_copy(ot, pv512)
            nc.sync.dma_start(out=out[ts(ct, 128), ds(q0 + tch * 512, 512)],
                              in_=ot)

        # mc1 out-proj is split by head pair: ft0 groups run inside windows
        # 6-7 (pair0 of mc1 is normalized by then) and DMA with bypass; ft1
        # groups run in the tail and accumulate into DRAM (AluOpType.add)
        ocp = out.rearrange("(cp c p) m -> cp p c m", c=2, p=128)
        ocp1 = out1.rearrange("(cp c p) m -> cp p c m", c=2, p=128)
        og_i = [0]

        def emit_og(ft, tch, cp):
            ot2 = dyn.tile([128, 2, 512], BF16, tag="os2", bufs=8,
                           name=f"ot2_{ft}_{tch}_{cp}")
            for cti in range(2):
                i = og_i[0]
                og_i[0] += 1
                tg = "op" if i % 2 == 0 else "sp"
                po = ps.tile([128, 512] if tg == "op" else [128, 1024], F32,
                             tag=tg, bufs=2, name=f"tpo{ft}_{tch}_{cp}_{cti}")
                pv512 = po[:, 0:512]
                nc.tensor.matmul(pv512, wo_sb[:, ft, ts(2 * cp + cti, 128)],
                                 AT[:, ft, ds(1024 + tch * 512, 512)],
                                 start=True, stop=True)
                dst = ot2[:, cti, :]
                if i % 2 == 0:
                    nc.scalar.copy(dst, pv512)
                else:
                    nc.vector.tensor_copy(dst, pv512)
            dv = ocp if ft == 0 else ocp1
            off = 1024 if ft == 0 else 0
            nc.sync.dma_start(out=dv[cp][:, :, ds(off + tch * 512, 512)],
                              in_=ot2)

        # ---- window schedule with sprinkled side-work ----
        windows = [(mci, h) for mci in range(2) for h in range(4)]

        # pre-window prologue: Q pair0 pieces 0..3 (cols 0:512, the first
        # half-units' moving data) and K piece 0 (keys 0:128); warm matmuls
        # bridge the x-piece DMA arrivals so the PE clock stays ramped
        emit_qproj(0, 0)
        emit_qproj(1, 0)
        emit_warm(18)
        emit_qproj(2, 0)
        emit_qproj(3, 0)
        emit_warm(6)
        emit_kproj(0)
        emit_warm(4)

        # per-window sprinkle thunks: (cost_ns, fn); kept <=~450ns each so
        # they slot into the PE slack between scores units
        QP = 430
        PV = 450
        OP = 450
        VP = 220

        def sprinkles(wi):
            mci, h = windows[wi]
            th = []
            if wi == 0:
                # kproj/ktdup are emitted inline with the units (ordering
                # constraint: PE is in-order, a score matmul emitted before
                # its K piece would stall the whole PE stream)
                th += [(VP, lambda tb=tb: emit_vproj(tb)) for tb in range(12)]
            else:
                pmci, ph = windows[wi - 1]
                nqb = MCS[pmci][1] // 128
                pv_th = [(PV, lambda qb=qb, m=pmci, hh=ph: emit_pv(m, hh, qb))
                         for qb in range(nqb)]
                if wi == 1:
                    # V tiles 12..15 must land before the first PV thunk
                    th += [(VP, lambda tb=tb: emit_vproj(tb))
                           for tb in range(12, 16)]
                    th += pv_th
                    th += [(QP, lambda j=j: emit_qproj(j, 1))
                           for j in range(8)]
                else:
                    th += pv_th
                if wi == 2:
                    th += [(QP, lambda j=j: emit_qproj(j, 0))
                           for j in range(8, 12)]
                    th += [(QP, lambda j=j: emit_qproj(j, 1))
                           for j in range(8, 12)]
                elif wi == 3:
                    th += [(QP, lambda j=j: emit_qproj(j, 0))
                           for j in range(12, 16)]
                    th += [(QP, lambda j=j: emit_qproj(j, 1))
                           for j in range(12, 16)]
                if wi in (5, 6):  # out-proj of mc0 during (1,h1)/(1,h2)
                    units = [(ct, tch) for tch in range(2) for ct in range(8)]
                    part = units[(wi - 5) * 8:(wi - 4) * 8]
                    th += [(OP, lambda u=u: emit_outproj(0, u[0], u[1]))
                           for u in part]
                if wi == 6:  # pair0 of mc1 out-proj, tch0 (query blocks 0-3)
                    th += [(OP, lambda cp=cp: emit_og(0, 0, cp))
                           for cp in range(4)]
                elif wi == 7:  # pair0 of mc1 out-proj, tch1
                    th += [(OP, lambda cp=cp: emit_og(0, 1, cp))
                           for cp in range(4)]
            return th

        # first-window unit order: the first 4 key tiles run as 512-wide
        # halves (the a-halves only need Q^T cols 0:512, i.e. the first two
        # x DMA pieces) so ACT starts early; the rest are full units, paced
        # well behind the x DMA stream
        W0_UNITS = ([(kb, 0) for kb in range(4)] + [(kb, 1) for kb in range(4)]
                    + [(kb, None) for kb in range(4, 16)])

        kp_done = set([0])
        qp1_done = [False]

        for wi, (mci, h) in enumerate(windows):
            P_of[(mci, h)] = dyn.tile([128, KB, 1024], BF16, tag="P", bufs=2,
                                      name=f"P{mci}_{h}")
            if h == 1:
                A_of[mci] = dyn.tile([128, MCS[mci][1] // 128, 256], BF16,
                                     tag="A", bufs=2, name=f"A{mci}")
            th = sprinkles(wi)
            total = sum(c for c, _ in th)
            spent = 0
            units = W0_UNITS if wi == 0 else [(kb, None) for kb in range(KB)]
            NU = len(units)
            for ui, (kb, half) in enumerate(units):
                if wi == 0:
                    if kb not in kp_done:
                        emit_kproj(kb)
                        kp_done.add(kb)
                        if len(kp_done) == KB:
                            emit_ktdup()
                    if (half == 1 or half is None) and not qp1_done[0]:
                        for j in range(4, 8):
                            emit_qproj(j, 0)
                        qp1_done[0] = True
                if half is None:
                    emit_scores_exp(mci, h, kb)
                else:
                    emit_scores_exp_half(mci, h, kb, half)
                # spread side work across the window, ~even by cost; finish
                # well before the window ends: the next window's first exp
                # WAR-waits on the previous PV reads, so late thunks stall ACT
                goal = min(total, total * (ui + 1) // (NU - 2))
                while th and spent < goal:
                    c, f = th.pop(0)
                    f()
                    spent += c
            while th:
                th.pop(0)[1]()

        # ---- tail: PV of last head interleaved with the pair1 half of
        # mc1's out-proj, which accumulates into DRAM on top of the pair0
        # half that was DMA'd out during windows 6-7.
        def tail_pv(qb, tag):
            emit_pv(1, 3, qb, tag=tag, chain="act" if qb % 2 == 0 else "dve")

        tail_pv(0, "pv")
        tail_pv(1, "pv")
        tail_pv(2, "sp")
        tail_pv(3, "sp")
        emit_og(1, 0, 0)
        emit_og(1, 0, 1)
        tail_pv(4, "pv")
        tail_pv(5, "pv")
        emit_og(1, 0, 2)
        emit_og(1, 0, 3)
        tail_pv(6, "sp")
        tail_pv(7, "sp")
        for cp in range(4):
            emit_og(1, 1, cp)

    nc.compile()
    return nc


_CACHE = {}


def _get_nc(NSEQ):
    if NSEQ not in _CACHE:
        _CACHE[NSEQ] = build_nc(NSEQ)
    return _CACHE[NSEQ]


def kernel(x, Wq, Wk, Wv, Wo, bo):
    """Full-input entry point: shard over 8 cores, run, gather."""
    import ml_dtypes
    bf16 = ml_dtypes.bfloat16
    x = np.asarray(x, np.float32)
    Wq = np.asarray(Wq, np.float32)
    Wk = np.asarray(Wk, np.float32)
    Wv = np.asarray(Wv, np.float32)
    Wo = np.asarray(Wo, np.float32)
    bo = np.asarray(bo, np.float32)
    B, N, C = x.shape
    nc = _get_nc(N)
    in_maps = []
    for c in range(8):
        b, g = c // 4, c % 4
        in_maps.append({
            "xT": np.ascontiguousarray(x[b].T).astype(bf16),
            "wq": np.ascontiguousarray(Wq[:, g * 256:(g + 1) * 256]).astype(bf16),
            "wk": np.ascontiguousarray(Wk[:, g * D:(g + 1) * D]).astype(bf16),
            "wv": np.ascontiguousarray(Wv[:, g * D:(g + 1) * D]).astype(bf16),
            "wo": np.ascontiguousarray(Wo[g * 256:(g + 1) * 256, :]).astype(bf16),
        })
    res = bass_utils.run_bass_kernel_spmd(nc, in_maps, core_ids=list(range(8)))
    outs = []
    for c in range(8):
        o = np.asarray(res.results[c]["out"]).astype(np.float32)
        o[:, N // 2:] += np.asarray(res.results[c]["out1"]).astype(np.float32)
        outs.append(o)
    full = np.empty((B, N, C), np.float32)
    for b in range(B):
        acc = outs[4 * b]
        for g in range(1, 4):
            acc = acc + outs[4 * b + g]
        full[b] = acc.T + bo[None, :]
    return full
